# revision 18
# baseline (speedup 1.0000x reference)
"""Trainium2 Bass kernel for an SNN layer (fc GEMM + leaky integrate-and-fire
scan + spike-count softmax), data-parallel over batch across 8 NeuronCores.

Computes, for x[64,128,4096], W[512,4096], b[512]:
    cur = einsum("bti,oi->bto", x, W) + b
    scan over t: mem' = 0.9*mem + cur_t - (mem > 1); spk_t = (mem' > 1)
    y = sum_t spk_t ; out = softmax(y, axis=-1)   -> [64, 512]

Strategy per core (batch shard of 8):
  - GEMM on PE in fp16 hi/lo split (3 passes, 1 cycle/row each vs fp32's
    4): x = xh + xl/2^11, W = Wh + Wl/2^11 with xh=fp16(x),
    xl=fp16((x-xh)*2^11) etc.  PSUM A accumulates xh*Wh; PSUM B
    accumulates xh*Wl + xl*Wh (both at scale 2^11).  Eviction computes
    S = A + B/2^11 + bias.  Dropped xl*wl term and fp16 rounding are
    ~2^-23 relative: measured end-to-end GEMM err 1.4e-7 — as exact as
    native fp32, so no spike flips vs the fp32 reference.
  - chunk 0 uses k-outer matmul order: each contraction tile is consumed
    in 12 matmuls right when its DMA lands, so compute streams behind the
    (W on sync-queue, x on act-queue) transfers with no bulk wait.
    Later chunks have everything resident and use c-outer for long
    uninterrupted PE runs (keeps the HAM clock gate at full rate).
  - cur^T tiles [no_part, rows], rows t-major/b-minor so the time scan
    reads contiguous 32-lane slices per step.
  - LIF scan: one fused custom-DVE op per timestep:
        mem_{t+1} = (mem_t * 0.9 + cur_t) - (mem_t > 1)
    over [128 part x 32 lanes], lanes = (no_chunk, b).
  - Spike count: (mem_traj > 1) then reduce over t, per chunk; softmax
    without max-subtraction (counts <= 128, use constant shift 64):
    PE-transpose y into one PSUM tile, single Exp eviction, reduce,
    reciprocal, scale.
"""

import numpy as np

B, T, NI, NO = 64, 128, 4096, 512
NCORES = 8
BS = B // NCORES            # 8 batch rows per core
KT = NI // 128              # 32 contraction tiles
CN = NO // 128              # 4 output chunks of 128
TSPLIT = [32, 32, 32, 32]   # timesteps per chunk
NH = len(TSPLIT)
TOFF = [sum(TSPLIT[:i]) for i in range(NH + 1)]
LAN = CN * BS               # 32 scan lanes per partition
BETA, THR = 0.9, 1.0
SC = float(2.0 ** 11)       # lo-piece scale
SHIFT = 64.0                # softmax constant shift (y in [0,128])
# k-tile DMA grouping: fine-grained leading groups so the first matmuls
# are gated on tiny transfers.
WGRP = [1, 1, 1, 1, 2, 2, 4, 4, 4, 4, 4, 4]
XGRP = [[1, 1, 1, 1, 2, 2, 4, 4, 4, 4, 4, 4]] + [[4] * 8] * (NH - 1)
assert sum(WGRP) == KT and all(sum(g) == KT for g in XGRP)


def _k2g(grp):
    m = {}
    k = 0
    for g, n in enumerate(grp):
        for j in range(n):
            m[k] = (g, j)
            k += 1
    return m


WK2G = _k2g(WGRP)
XK2G = [_k2g(g) for g in XGRP]
# flat x layout: per (chunk h, group g) a [128, n_g*2*RH_h] fp16 block,
# row-major; within a group, k-tile j's hi piece at (2j)*rh, lo at (2j+1)*rh
XSIZES = [128 * n * 2 * (TSPLIT[h] * BS) for h in range(NH) for n in XGRP[h]]
XOFFS = [sum(XSIZES[:i]) for i in range(len(XSIZES) + 1)]
XTOT = XOFFS[-1]
XBLK = [sum(len(XGRP[i]) for i in range(h)) for h in range(NH)]

_PROG = None


def _lif_op():
    """Register (idempotently) the fused LIF-step custom DVE op:
    out = (Src0 * C0 + Src1) - (Src0 > C1)."""
    from concourse import dve_ops
    from concourse.dve_ops import DveOp
    from concourse.dve_spec import Spec, Src0, Src1, C0, C1, lower, _has_src1
    from concourse.dve_uop import DveOpSpec

    name = "LIF_STEP_ANT"
    for op in dve_ops.OPS:
        if op.name == name:
            return op

    spec = Spec(
        body=(Src0 * C0 + Src1) - (Src0 > C1),
        reference=lambda in0, in1, s0, s1, imm2: (
            (in0.astype(np.float32) * np.float32(s0) + in1)
            - (in0 > s1).astype(np.float32)
        ),
    )
    row = dve_ops._CUSTOM_DVE_ROW_BASE + len(dve_ops.OPS)
    assert row < 0x20, "custom DVE opcode rows exhausted"
    dve_ops._SUB_OPCODE_FOR_NAME[name] = row
    shas = {}
    for ver in ("v3", "v4"):
        uops = lower(spec, ver=ver)
        shas[ver] = DveOpSpec(
            name=name, opcode=row, uops=uops, rd1_en=_has_src1(spec)
        ).sha(ver)
    op = DveOp(name, spec, subdim=False, uops_sha=shas)
    dve_ops.OPS.append(op)
    dve_ops.CUSTOM_DVE_SPECS[name] = spec
    return op


def build_program():
    import concourse.bacc as bacc
    import concourse.mybir as mybir
    from concourse import tile
    from concourse.masks import make_identity

    f32 = mybir.dt.float32
    f16 = mybir.dt.float16
    lif = _lif_op()

    nc = bacc.Bacc("TRN2", target_bir_lowering=False, debug=False)

    xt_d = nc.dram_tensor("xt", [XTOT], f16, kind="ExternalInput").ap()
    # per k-tile: Wh_k [128, NO] then Wl_k [128, NO]
    wt_d = nc.dram_tensor("wt", [128, KT * 2 * NO], f16,
                          kind="ExternalInput").ap()
    bt_d = nc.dram_tensor("bt", [CN, 128], f32, kind="ExternalInput").ap()
    out_d = nc.dram_tensor("out", [BS, NO], f32, kind="ExternalOutput").ap()

    def x_dram(h, g):
        i = XBLK[h] + g
        return xt_d[XOFFS[i]:XOFFS[i + 1]].rearrange("(p q) -> p q", p=128)

    with tile.TileContext(nc) as tc:
        with (
            tc.tile_pool(name="wp", bufs=1) as wp,
            tc.tile_pool(name="xp", bufs=20) as xp,
            tc.tile_pool(name="sp", bufs=1) as sp,
            tc.tile_pool(name="cp", bufs=1) as cp,
            tc.tile_pool(name="tmp", bufs=2) as tmpp,
            tc.tile_pool(name="accp", bufs=1, space="PSUM") as accp,
        ):
            # --- PE warmup: burn the HAM clock-gate ramp during the
            # fixed ~10us prologue + first-DMA latency window, so real
            # matmuls start at full rate.  Values are junk; the real
            # accumulation's start=True overwrites the bank. -----------
            warm = wp.tile([128, 256], f16, name="warm", tag="warm")
            nc.gpsimd.memset(warm[:], 0.0)
            wps = accp.tile([128, 512], f32, name="warm_ps", tag="accA0")
            for _ in range(12):
                nc.tensor.matmul(wps[:, 0:256], lhsT=warm[:, 0:128],
                                 rhs=warm[:], start=True, stop=True)

            # --- W (sync queue) / x (act queue) DMAs --------------------
            wg = [wp.tile([128, n * 2 * NO], f16, name=f"wg{g}", tag=f"wg{g}")
                  for g, n in enumerate(WGRP)]
            xg_tiles = {}

            def issue_x(h, g):
                n = XGRP[h][g]
                t_ = xp.tile([128, n * 2 * TSPLIT[h] * BS], f16,
                             name=f"xg{h}_{g}", tag="xg",
                             padded_shape=[128, 4 * 2 * max(TSPLIT) * BS])
                nc.scalar.dma_start(out=t_[:], in_=x_dram(h, g))
                xg_tiles[(h, g)] = t_

            WOFF = [sum(WGRP[:i]) for i in range(len(WGRP))]

            def issue_w(g):
                nc.sync.dma_start(
                    out=wg[g][:],
                    in_=wt_d[:, WOFF[g] * 2 * NO:(WOFF[g] + WGRP[g]) * 2 * NO])

            for g in range(max(len(WGRP), len(XGRP[0]))):
                if g < len(WGRP):
                    issue_w(g)
                if g < len(XGRP[0]):
                    issue_x(0, g)

            # bias as one value per partition: b_sb[c][p] = b[c*128+p]
            b_sb = [cp.tile([128, 1], f32, name=f"bsb{c}", tag=f"bsb{c}")
                    for c in range(CN)]
            for c in range(CN):
                nc.sync.dma_start(out=b_sb[c][:],
                                  in_=bt_d[c:c + 1, :].rearrange("a p -> p a"))
            zer = cp.tile([128, LAN], f32, name="zer", tag="zer")
            nc.gpsimd.memset(zer[:], 0.0)
            ident = cp.tile([128, 128], f32, name="ident", tag="ident")
            make_identity(nc, ident[:])

            S = [sp.tile([128, TSPLIT[h] * LAN], f32, name=f"s{h}", tag=f"s{h}")
                 for h in range(NH)]
            M = [sp.tile([128, TSPLIT[h] * LAN], f32, name=f"m{h}", tag=f"m{h}")
                 for h in range(NH)]
            yh = [cp.tile([128, LAN], f32, name=f"yh{h}", tag=f"yh{h}")
                  for h in range(NH)]
            ysum = cp.tile([128, LAN], f32, name="ysum", tag="ysum")

            def w_ap(wgt, wj, piece, c):
                off = wj * 2 * NO + piece * NO + c * 128
                return wgt[:, off:off + 128]

            def x_ap(xt_t, xj, piece, rh):
                off = (2 * xj + piece) * rh
                return xt_t[:, off:off + rh]

            # --- GEMM + scan + count, pipelined over time chunks --------
            for h in range(NH):
                ts = TSPLIT[h]
                rh = ts * BS
                # prefetch next chunk's x groups
                if h + 1 < NH:
                    for g in range(len(XGRP[h + 1])):
                        issue_x(h + 1, g)
                accA = [accp.tile([128, rh], f32, name=f"accA{h}_{c}",
                                  tag=f"accA{c}", padded_shape=[128, 512])
                        for c in range(CN)]
                accB = [accp.tile([128, rh], f32, name=f"accB{h}_{c}",
                                  tag=f"accB{c}", padded_shape=[128, 512])
                        for c in range(CN)]

                def mm3(c, k):
                    xgi, xj = XK2G[h][k]
                    wgi, wj = WK2G[k]
                    xt_t = xg_tiles[(h, xgi)]
                    wgt = wg[wgi]
                    xh_ap = x_ap(xt_t, xj, 0, rh)
                    xl_ap = x_ap(xt_t, xj, 1, rh)
                    nc.tensor.matmul(
                        accA[c][:], lhsT=w_ap(wgt, wj, 0, c), rhs=xh_ap,
                        start=(k == 0), stop=(k == KT - 1))
                    nc.tensor.matmul(
                        accB[c][:], lhsT=w_ap(wgt, wj, 0, c), rhs=xl_ap,
                        start=(k == 0), stop=False)
                    nc.tensor.matmul(
                        accB[c][:], lhsT=w_ap(wgt, wj, 1, c), rhs=xh_ap,
                        start=False, stop=(k == KT - 1))

                if h == 0:
                    # W/x stream in during chunk 0: k-outer consumes each
                    # k-tile in 12 matmuls right as its DMA lands, so the
                    # PE tracks the transfers instead of bulk-waiting.
                    for k in range(KT):
                        for c in range(CN):
                            mm3(c, k)
                else:
                    # everything resident: c-outer gives long PE runs
                    for c in range(CN):
                        for k in range(KT):
                            mm3(c, k)
                # evict: S[., ., c] = A + B/SC + bias_c
                s_v = S[h].rearrange("p (t l) -> p t l", l=LAN)
                for c in range(CN):
                    tmp = tmpp.tile([128, rh], f32, name=f"tmp{h}_{c}",
                                    tag="tmp", padded_shape=[128, 256])
                    nc.scalar.activation(
                        tmp[:], accB[c][:],
                        mybir.ActivationFunctionType.Identity,
                        bias=b_sb[c][:], scale=1.0 / SC)
                    nc.vector.tensor_tensor(
                        out=s_v[:, :, c * BS:(c + 1) * BS],
                        in0=accA[c][:].rearrange("p (t b) -> p t b", b=BS),
                        in1=tmp[:].rearrange("p (t b) -> p t b", b=BS),
                        op=mybir.AluOpType.add)
                # LIF scan for this chunk's timesteps
                for tt in range(ts):
                    t = TOFF[h] + tt
                    cur = S[h][:, tt * LAN:(tt + 1) * LAN]
                    dst = M[h][:, tt * LAN:(tt + 1) * LAN]
                    if t == 0:
                        prev = zer[:]
                    elif tt == 0:
                        prev = M[h - 1][:, (TSPLIT[h - 1] - 1) * LAN:
                                        TSPLIT[h - 1] * LAN]
                    else:
                        prev = M[h][:, (tt - 1) * LAN: tt * LAN]
                    nc.vector._custom_dve(lif, out=dst, in0=prev, in1=cur,
                                          s0=BETA, s1=THR)
                # spike count for this chunk (overlaps next chunk's GEMM);
                # S[h]/M[h] are dead after the scan -> reuse as scratch.
                # Contiguous halving tree over t beats the strided reduce
                # (t-major layout pairs equal lanes at each halving).
                nc.vector.tensor_scalar(out=S[h][:], in0=M[h][:], scalar1=THR,
                                        scalar2=None, op0=mybir.AluOpType.is_gt)
                srcs = [S[h], M[h]]
                n = ts * LAN
                si = 0
                while n > LAN:
                    half = n // 2
                    a, b = srcs[si], srcs[1 - si]
                    dst = yh[h][:] if half == LAN else b[:, 0:half]
                    nc.vector.tensor_tensor(out=dst, in0=a[:, 0:half],
                                            in1=a[:, half:n],
                                            op=mybir.AluOpType.add)
                    n = half
                    si = 1 - si
                # incremental y accumulation (hidden under later GEMM)
                if h == 1:
                    nc.vector.tensor_tensor(
                        out=ysum[:], in0=yh[0][:], in1=yh[1][:],
                        op=mybir.AluOpType.add)
                elif h >= 2:
                    nc.vector.tensor_tensor(
                        out=ysum[:], in0=ysum[:], in1=yh[h][:],
                        op=mybir.AluOpType.add)

            # --- transpose y^T [no, b] -> one PSUM bank [b, no] ---------
            # (reuses the accA0 bank; each 128-col slice written once, so
            # later start-flag clears don't disturb earlier values)
            ytile = accp.tile([128, 512], f32, name="ytp", tag="accA0")
            ytp = ytile[0:BS, 0:NO]
            y_v = ysum.rearrange("p (c b) -> p c b", b=BS)
            for c in range(CN):
                nc.tensor.transpose(ytp[:, c * 128:(c + 1) * 128],
                                    y_v[:, c, :], ident[:])

            # --- softmax over no (free dim); no max needed: y in [0,128],
            # softmax(y) == softmax(y - SHIFT) exactly ---------------------
            nsh = cp.tile([BS, 1], f32, name="nsh", tag="nsh")
            nc.gpsimd.memset(nsh[:], -SHIFT)
            ex = cp.tile([BS, NO], f32, name="ex", tag="ex")
            nc.scalar.activation(ex[:], ytp[:],
                                 mybir.ActivationFunctionType.Exp,
                                 bias=nsh[:], scale=1.0)
            sm = cp.tile([BS, 1], f32, name="sm", tag="sm")
            nc.vector.tensor_reduce(out=sm[:], in_=ex[:],
                                    axis=mybir.AxisListType.X,
                                    op=mybir.AluOpType.add)
            rc = cp.tile([BS, 1], f32, name="rc", tag="rc")
            nc.vector.reciprocal(rc[:], sm[:])
            res = cp.tile([BS, NO], f32, name="res", tag="res")
            nc.vector.tensor_scalar(out=res[:], in0=ex[:], scalar1=rc[:],
                                    scalar2=None, op0=mybir.AluOpType.mult)

            nc.sync.dma_start(out=out_d[:], in_=res[:])

    nc.compile()
    return nc


def prep_inputs(x, W, b):
    """Host-side layout prep. Returns per-core in_maps."""
    x = np.asarray(x, dtype=np.float32)
    W = np.asarray(W, dtype=np.float32)
    b = np.asarray(b, dtype=np.float32)

    # fp16 hi/lo splits (lo pre-scaled by SC; exact residuals)
    Wh = W.astype(np.float16)
    Wl = ((W - Wh.astype(np.float32)) * SC).astype(np.float16)
    xh = x.astype(np.float16)
    xl = ((x - xh.astype(np.float32)) * SC).astype(np.float16)

    # wbig[p, k*2*NO + piece*NO + j] = Wpiece[j, k*128 + p]
    def wlayout(Wp):
        return np.ascontiguousarray(
            Wp.T.reshape(KT, 128, NO).transpose(1, 0, 2))  # [128, KT, NO]

    wb = np.empty((128, KT, 2, NO), np.float16)
    wb[:, :, 0, :] = wlayout(Wh)
    wb[:, :, 1, :] = wlayout(Wl)
    wbig = np.ascontiguousarray(wb.reshape(128, KT * 2 * NO))
    bt = np.ascontiguousarray(b.reshape(CN, 128))

    # x flat layout: blocks (h, g) of [128, n_g, 2, RH_h], r = tt*BS + b_loc
    xTh = np.ascontiguousarray(xh.transpose(2, 1, 0))  # [NI, T, B] fp16
    xTl = np.ascontiguousarray(xl.transpose(2, 1, 0))
    in_maps = []
    for ci in range(NCORES):
        bsl = slice(ci * BS, (ci + 1) * BS)
        xkh = xTh[:, :, bsl].reshape(KT, 128, T * BS)   # [k, p, t*BS+b]
        xkl = xTl[:, :, bsl].reshape(KT, 128, T * BS)
        flat = np.empty(XTOT, np.float16)
        i = 0
        for h in range(NH):
            r0, r1 = TOFF[h] * BS, TOFF[h + 1] * BS
            rh = r1 - r0
            k0 = 0
            for n_g in XGRP[h]:
                blk = np.empty((128, n_g, 2, rh), np.float16)
                blk[:, :, 0, :] = xkh[k0:k0 + n_g, :, r0:r1].transpose(1, 0, 2)
                blk[:, :, 1, :] = xkl[k0:k0 + n_g, :, r0:r1].transpose(1, 0, 2)
                flat[XOFFS[i]:XOFFS[i] + blk.size] = blk.reshape(-1)
                k0 += n_g
                i += 1
        in_maps.append({"xt": flat, "wt": wbig, "bt": bt})
    return in_maps


def get_program():
    global _PROG
    if _PROG is None:
        _PROG = build_program()
    return _PROG


def kernel(x, W, b):
    from concourse import bass_utils

    nc = get_program()
    in_maps = prep_inputs(x, W, b)
    res = bass_utils.run_bass_kernel_spmd(nc, in_maps,
                                          core_ids=list(range(NCORES)))
    return np.concatenate([res.results[i]["out"] for i in range(NCORES)],
                          axis=0)


# revision 19
# speedup vs baseline: 1.0461x; 1.0461x over previous
"""Trainium2 Bass kernel for an SNN layer (fc GEMM + leaky integrate-and-fire
scan + spike-count softmax), data-parallel over batch across 8 NeuronCores.

Computes, for x[64,128,4096], W[512,4096], b[512]:
    cur = einsum("bti,oi->bto", x, W) + b
    scan over t: mem' = 0.9*mem + cur_t - (mem > 1); spk_t = (mem' > 1)
    y = sum_t spk_t ; out = softmax(y, axis=-1)   -> [64, 512]

Strategy per core (batch shard of 8):
  - GEMM on PE in fp16 hi/lo split (3 passes, 1 cycle/row each vs fp32's
    4): x = xh + xl/2^11, W = Wh + Wl/2^11 with xh=fp16(x),
    xl=fp16((x-xh)*2^11) etc.  PSUM A accumulates xh*Wh; PSUM B
    accumulates xh*Wl + xl*Wh (both at scale 2^11).  Eviction computes
    S = A + B/2^11 + bias.  Dropped xl*wl term and fp16 rounding are
    ~2^-23 relative: measured end-to-end GEMM err 1.4e-7 — as exact as
    native fp32, so no spike flips vs the fp32 reference.
  - chunk 0 uses k-outer matmul order: each contraction tile is consumed
    in 12 matmuls right when its DMA lands, so compute streams behind the
    (W on sync-queue, x on act-queue) transfers with no bulk wait.
    Later chunks have everything resident and use c-outer for long
    uninterrupted PE runs (keeps the HAM clock gate at full rate).
  - cur^T tiles [no_part, rows], rows t-major/b-minor so the time scan
    reads contiguous 32-lane slices per step.
  - LIF scan: one fused custom-DVE op per timestep:
        mem_{t+1} = (mem_t * 0.9 + cur_t) - (mem_t > 1)
    over [128 part x 32 lanes], lanes = (no_chunk, b).
  - Spike count: (mem_traj > 1) then reduce over t, per chunk; softmax
    without max-subtraction (counts <= 128, use constant shift 64):
    PE-transpose y into one PSUM tile, single Exp eviction, reduce,
    reciprocal, scale.
"""

import numpy as np

B, T, NI, NO = 64, 128, 4096, 512
NCORES = 8
BS = B // NCORES            # 8 batch rows per core
KT = NI // 128              # 32 contraction tiles
CN = NO // 128              # 4 output chunks of 128
TSPLIT = [32, 32, 32, 32]   # timesteps per chunk
NH = len(TSPLIT)
TOFF = [sum(TSPLIT[:i]) for i in range(NH + 1)]
LAN = CN * BS               # 32 scan lanes per partition
BETA, THR = 0.9, 1.0
SC = float(2.0 ** 11)       # lo-piece scale
SHIFT = 64.0                # softmax constant shift (y in [0,128])
# k-tile DMA grouping: fine-grained leading groups so the first matmuls
# are gated on tiny transfers.
WGRP = [1, 1, 1, 1, 2, 2, 4, 4, 4, 4, 4, 4]
XGRP = [[1, 1, 1, 1, 2, 2, 4, 4, 4, 4, 4, 4]] + [[4] * 8] * (NH - 1)
assert sum(WGRP) == KT and all(sum(g) == KT for g in XGRP)


def _k2g(grp):
    m = {}
    k = 0
    for g, n in enumerate(grp):
        for j in range(n):
            m[k] = (g, j)
            k += 1
    return m


WK2G = _k2g(WGRP)
XK2G = [_k2g(g) for g in XGRP]
# flat x layout: per (chunk h, group g) a [128, n_g*2*RH_h] fp16 block,
# row-major; within a group, k-tile j's hi piece at (2j)*rh, lo at (2j+1)*rh
XSIZES = [128 * n * 2 * (TSPLIT[h] * BS) for h in range(NH) for n in XGRP[h]]
XOFFS = [sum(XSIZES[:i]) for i in range(len(XSIZES) + 1)]
XTOT = XOFFS[-1]
XBLK = [sum(len(XGRP[i]) for i in range(h)) for h in range(NH)]

_PROG = None


def _lif_op():
    """Register (idempotently) the fused LIF-step custom DVE op:
    out = (Src0 * C0 + Src1) - (Src0 > C1)."""
    from concourse import dve_ops
    from concourse.dve_ops import DveOp
    from concourse.dve_spec import Spec, Src0, Src1, C0, C1, lower, _has_src1
    from concourse.dve_uop import DveOpSpec

    name = "LIF_STEP_ANT"
    for op in dve_ops.OPS:
        if op.name == name:
            return op

    spec = Spec(
        body=(Src0 * C0 + Src1) - (Src0 > C1),
        reference=lambda in0, in1, s0, s1, imm2: (
            (in0.astype(np.float32) * np.float32(s0) + in1)
            - (in0 > s1).astype(np.float32)
        ),
    )
    row = dve_ops._CUSTOM_DVE_ROW_BASE + len(dve_ops.OPS)
    assert row < 0x20, "custom DVE opcode rows exhausted"
    dve_ops._SUB_OPCODE_FOR_NAME[name] = row
    shas = {}
    for ver in ("v3", "v4"):
        uops = lower(spec, ver=ver)
        shas[ver] = DveOpSpec(
            name=name, opcode=row, uops=uops, rd1_en=_has_src1(spec)
        ).sha(ver)
    op = DveOp(name, spec, subdim=False, uops_sha=shas)
    dve_ops.OPS.append(op)
    dve_ops.CUSTOM_DVE_SPECS[name] = spec
    return op


def build_program():
    import concourse.bacc as bacc
    import concourse.mybir as mybir
    from concourse import tile
    from concourse.masks import make_identity

    f32 = mybir.dt.float32
    f16 = mybir.dt.float16
    lif = _lif_op()

    nc = bacc.Bacc("TRN2", target_bir_lowering=False, debug=False)

    xt_d = nc.dram_tensor("xt", [XTOT], f16, kind="ExternalInput").ap()
    # per k-tile: Wh_k [128, NO] then Wl_k [128, NO]
    wt_d = nc.dram_tensor("wt", [128, KT * 2 * NO], f16,
                          kind="ExternalInput").ap()
    bt_d = nc.dram_tensor("bt", [CN, 128], f32, kind="ExternalInput").ap()
    out_d = nc.dram_tensor("out", [BS, NO], f32, kind="ExternalOutput").ap()

    def x_dram(h, g):
        i = XBLK[h] + g
        return xt_d[XOFFS[i]:XOFFS[i + 1]].rearrange("(p q) -> p q", p=128)

    with tile.TileContext(nc) as tc:
        with (
            tc.tile_pool(name="wp", bufs=1) as wp,
            tc.tile_pool(name="xp", bufs=12) as xp,
            tc.tile_pool(name="sp", bufs=1) as sp,
            tc.tile_pool(name="cp", bufs=1) as cp,
            tc.tile_pool(name="tmp", bufs=2) as tmpp,
            tc.tile_pool(name="accp", bufs=1, space="PSUM") as accp,
        ):
            # --- PE warmup: burn the HAM clock-gate ramp during the
            # fixed ~10us prologue + first-DMA latency window, so real
            # matmuls start at full rate.  Values are junk; the real
            # accumulation's start=True overwrites the bank. -----------
            warm = wp.tile([128, 256], f16, name="warm", tag="warm")
            nc.gpsimd.memset(warm[:], 0.0)
            wps = accp.tile([128, 512], f32, name="warm_ps", tag="accA0")
            for _ in range(12):
                nc.tensor.matmul(wps[:, 0:256], lhsT=warm[:, 0:128],
                                 rhs=warm[:], start=True, stop=True)

            # --- W (sync queue) / x (act queue) DMAs --------------------
            wg = [wp.tile([128, n * 2 * NO], f16, name=f"wg{g}", tag=f"wg{g}")
                  for g, n in enumerate(WGRP)]
            xg_tiles = {}

            def issue_x(h, g):
                n = XGRP[h][g]
                t_ = xp.tile([128, n * 2 * TSPLIT[h] * BS], f16,
                             name=f"xg{h}_{g}", tag="xg",
                             padded_shape=[128, 4 * 2 * max(TSPLIT) * BS])
                nc.scalar.dma_start(out=t_[:], in_=x_dram(h, g))
                xg_tiles[(h, g)] = t_

            WOFF = [sum(WGRP[:i]) for i in range(len(WGRP))]

            def issue_w(g):
                nc.sync.dma_start(
                    out=wg[g][:],
                    in_=wt_d[:, WOFF[g] * 2 * NO:(WOFF[g] + WGRP[g]) * 2 * NO])

            for g in range(max(len(WGRP), len(XGRP[0]))):
                if g < len(WGRP):
                    issue_w(g)
                if g < len(XGRP[0]):
                    issue_x(0, g)

            # bias as one value per partition: b_sb[c][p] = b[c*128+p]
            b_sb = [cp.tile([128, 1], f32, name=f"bsb{c}", tag=f"bsb{c}")
                    for c in range(CN)]
            for c in range(CN):
                nc.sync.dma_start(out=b_sb[c][:],
                                  in_=bt_d[c:c + 1, :].rearrange("a p -> p a"))
            zer = cp.tile([128, LAN], f32, name="zer", tag="zer")
            nc.gpsimd.memset(zer[:], 0.0)
            ident = cp.tile([128, 128], f32, name="ident", tag="ident")
            make_identity(nc, ident[:])

            S = [sp.tile([128, TSPLIT[h] * LAN], f32, name=f"s{h}", tag=f"s{h}")
                 for h in range(NH)]
            M = [sp.tile([128, TSPLIT[h] * LAN], f32, name=f"m{h}", tag=f"m{h}")
                 for h in range(NH)]
            yh = [cp.tile([128, LAN], f32, name=f"yh{h}", tag=f"yh{h}")
                  for h in range(NH)]
            ysum = cp.tile([128, LAN], f32, name="ysum", tag="ysum")

            def w_ap(wgt, wj, piece, c):
                off = wj * 2 * NO + piece * NO + c * 128
                return wgt[:, off:off + 128]

            def x_ap(xt_t, xj, piece, rh):
                off = (2 * xj + piece) * rh
                return xt_t[:, off:off + rh]

            # --- GEMM + scan + count, pipelined over time chunks --------
            for h in range(NH):
                ts = TSPLIT[h]
                rh = ts * BS
                # prefetch next chunk's x groups
                if h + 1 < NH:
                    for g in range(len(XGRP[h + 1])):
                        issue_x(h + 1, g)
                accA = [accp.tile([128, rh], f32, name=f"accA{h}_{c}",
                                  tag=f"accA{c}", padded_shape=[128, 512])
                        for c in range(CN)]
                accB = [accp.tile([128, rh], f32, name=f"accB{h}_{c}",
                                  tag=f"accB{c}", padded_shape=[128, 512])
                        for c in range(CN)]

                def mm3(c, k):
                    xgi, xj = XK2G[h][k]
                    wgi, wj = WK2G[k]
                    xt_t = xg_tiles[(h, xgi)]
                    wgt = wg[wgi]
                    xh_ap = x_ap(xt_t, xj, 0, rh)
                    xl_ap = x_ap(xt_t, xj, 1, rh)
                    nc.tensor.matmul(
                        accA[c][:], lhsT=w_ap(wgt, wj, 0, c), rhs=xh_ap,
                        start=(k == 0), stop=(k == KT - 1))
                    nc.tensor.matmul(
                        accB[c][:], lhsT=w_ap(wgt, wj, 0, c), rhs=xl_ap,
                        start=(k == 0), stop=False)
                    nc.tensor.matmul(
                        accB[c][:], lhsT=w_ap(wgt, wj, 1, c), rhs=xh_ap,
                        start=False, stop=(k == KT - 1))

                if h == 0:
                    # W/x stream in during chunk 0: k-outer consumes each
                    # k-tile in 12 matmuls right as its DMA lands, so the
                    # PE tracks the transfers instead of bulk-waiting.
                    for k in range(KT):
                        for c in range(CN):
                            mm3(c, k)
                else:
                    # everything resident: c-outer gives long PE runs
                    for c in range(CN):
                        for k in range(KT):
                            mm3(c, k)
                # evict: S[., ., c] = A + B/SC + bias_c
                s_v = S[h].rearrange("p (t l) -> p t l", l=LAN)
                for c in range(CN):
                    tmp = tmpp.tile([128, rh], f32, name=f"tmp{h}_{c}",
                                    tag="tmp", padded_shape=[128, 256])
                    nc.scalar.activation(
                        tmp[:], accB[c][:],
                        mybir.ActivationFunctionType.Identity,
                        bias=b_sb[c][:], scale=1.0 / SC)
                    nc.vector.tensor_tensor(
                        out=s_v[:, :, c * BS:(c + 1) * BS],
                        in0=accA[c][:].rearrange("p (t b) -> p t b", b=BS),
                        in1=tmp[:].rearrange("p (t b) -> p t b", b=BS),
                        op=mybir.AluOpType.add)
                # LIF scan for this chunk's timesteps
                for tt in range(ts):
                    t = TOFF[h] + tt
                    cur = S[h][:, tt * LAN:(tt + 1) * LAN]
                    dst = M[h][:, tt * LAN:(tt + 1) * LAN]
                    if t == 0:
                        prev = zer[:]
                    elif tt == 0:
                        prev = M[h - 1][:, (TSPLIT[h - 1] - 1) * LAN:
                                        TSPLIT[h - 1] * LAN]
                    else:
                        prev = M[h][:, (tt - 1) * LAN: tt * LAN]
                    nc.vector._custom_dve(lif, out=dst, in0=prev, in1=cur,
                                          s0=BETA, s1=THR)
                # spike count for this chunk (overlaps next chunk's GEMM);
                # S[h]/M[h] are dead after the scan -> reuse as scratch.
                # Contiguous halving tree over t beats the strided reduce
                # (t-major layout pairs equal lanes at each halving).
                nc.vector.tensor_scalar(out=S[h][:], in0=M[h][:], scalar1=THR,
                                        scalar2=None, op0=mybir.AluOpType.is_gt)
                srcs = [S[h], M[h]]
                n = ts * LAN
                si = 0
                while n > LAN:
                    half = n // 2
                    a, b = srcs[si], srcs[1 - si]
                    dst = yh[h][:] if half == LAN else b[:, 0:half]
                    nc.vector.tensor_tensor(out=dst, in0=a[:, 0:half],
                                            in1=a[:, half:n],
                                            op=mybir.AluOpType.add)
                    n = half
                    si = 1 - si
                # incremental y accumulation (hidden under later GEMM)
                if h == 1:
                    nc.vector.tensor_tensor(
                        out=ysum[:], in0=yh[0][:], in1=yh[1][:],
                        op=mybir.AluOpType.add)
                elif h >= 2:
                    nc.vector.tensor_tensor(
                        out=ysum[:], in0=ysum[:], in1=yh[h][:],
                        op=mybir.AluOpType.add)

            # --- transpose y^T [no, b] -> one PSUM bank [b, no] ---------
            # (reuses the accA0 bank; each 128-col slice written once, so
            # later start-flag clears don't disturb earlier values)
            ytile = accp.tile([128, 512], f32, name="ytp", tag="accA0")
            ytp = ytile[0:BS, 0:NO]
            y_v = ysum.rearrange("p (c b) -> p c b", b=BS)
            for c in range(CN):
                nc.tensor.transpose(ytp[:, c * 128:(c + 1) * 128],
                                    y_v[:, c, :], ident[:])

            # --- softmax over no (free dim); no max needed: y in [0,128],
            # softmax(y) == softmax(y - SHIFT) exactly ---------------------
            nsh = cp.tile([BS, 1], f32, name="nsh", tag="nsh")
            nc.gpsimd.memset(nsh[:], -SHIFT)
            ex = cp.tile([BS, NO], f32, name="ex", tag="ex")
            nc.scalar.activation(ex[:], ytp[:],
                                 mybir.ActivationFunctionType.Exp,
                                 bias=nsh[:], scale=1.0)
            sm = cp.tile([BS, 1], f32, name="sm", tag="sm")
            nc.vector.tensor_reduce(out=sm[:], in_=ex[:],
                                    axis=mybir.AxisListType.X,
                                    op=mybir.AluOpType.add)
            rc = cp.tile([BS, 1], f32, name="rc", tag="rc")
            nc.vector.reciprocal(rc[:], sm[:])
            res = cp.tile([BS, NO], f32, name="res", tag="res")
            nc.vector.tensor_scalar(out=res[:], in0=ex[:], scalar1=rc[:],
                                    scalar2=None, op0=mybir.AluOpType.mult)

            nc.sync.dma_start(out=out_d[:], in_=res[:])

    nc.compile()
    return nc


def prep_inputs(x, W, b):
    """Host-side layout prep. Returns per-core in_maps."""
    x = np.asarray(x, dtype=np.float32)
    W = np.asarray(W, dtype=np.float32)
    b = np.asarray(b, dtype=np.float32)

    # fp16 hi/lo splits (lo pre-scaled by SC; exact residuals)
    Wh = W.astype(np.float16)
    Wl = ((W - Wh.astype(np.float32)) * SC).astype(np.float16)
    xh = x.astype(np.float16)
    xl = ((x - xh.astype(np.float32)) * SC).astype(np.float16)

    # wbig[p, k*2*NO + piece*NO + j] = Wpiece[j, k*128 + p]
    def wlayout(Wp):
        return np.ascontiguousarray(
            Wp.T.reshape(KT, 128, NO).transpose(1, 0, 2))  # [128, KT, NO]

    wb = np.empty((128, KT, 2, NO), np.float16)
    wb[:, :, 0, :] = wlayout(Wh)
    wb[:, :, 1, :] = wlayout(Wl)
    wbig = np.ascontiguousarray(wb.reshape(128, KT * 2 * NO))
    bt = np.ascontiguousarray(b.reshape(CN, 128))

    # x flat layout: blocks (h, g) of [128, n_g, 2, RH_h], r = tt*BS + b_loc
    xTh = np.ascontiguousarray(xh.transpose(2, 1, 0))  # [NI, T, B] fp16
    xTl = np.ascontiguousarray(xl.transpose(2, 1, 0))
    in_maps = []
    for ci in range(NCORES):
        bsl = slice(ci * BS, (ci + 1) * BS)
        xkh = xTh[:, :, bsl].reshape(KT, 128, T * BS)   # [k, p, t*BS+b]
        xkl = xTl[:, :, bsl].reshape(KT, 128, T * BS)
        flat = np.empty(XTOT, np.float16)
        i = 0
        for h in range(NH):
            r0, r1 = TOFF[h] * BS, TOFF[h + 1] * BS
            rh = r1 - r0
            k0 = 0
            for n_g in XGRP[h]:
                blk = np.empty((128, n_g, 2, rh), np.float16)
                blk[:, :, 0, :] = xkh[k0:k0 + n_g, :, r0:r1].transpose(1, 0, 2)
                blk[:, :, 1, :] = xkl[k0:k0 + n_g, :, r0:r1].transpose(1, 0, 2)
                flat[XOFFS[i]:XOFFS[i] + blk.size] = blk.reshape(-1)
                k0 += n_g
                i += 1
        in_maps.append({"xt": flat, "wt": wbig, "bt": bt})
    return in_maps


def get_program():
    global _PROG
    if _PROG is None:
        _PROG = build_program()
    return _PROG


def kernel(x, W, b):
    from concourse import bass_utils

    nc = get_program()
    in_maps = prep_inputs(x, W, b)
    res = bass_utils.run_bass_kernel_spmd(nc, in_maps,
                                          core_ids=list(range(NCORES)))
    return np.concatenate([res.results[i]["out"] for i in range(NCORES)],
                          axis=0)


# revision 26
# speedup vs baseline: 1.0560x; 1.0094x over previous
"""Trainium2 Bass kernel for an SNN layer (fc GEMM + leaky integrate-and-fire
scan + spike-count softmax), data-parallel over batch across 8 NeuronCores.

Computes, for x[64,128,4096], W[512,4096], b[512]:
    cur = einsum("bti,oi->bto", x, W) + b
    scan over t: mem' = 0.9*mem + cur_t - (mem > 1); spk_t = (mem' > 1)
    y = sum_t spk_t ; out = softmax(y, axis=-1)   -> [64, 512]

Strategy per core (batch shard of 8):
  - GEMM on PE in fp16 hi/lo split (3 passes, 1 cycle/row each vs fp32's
    4): x = xh + xl/2^11, W = Wh + Wl/2^11 with xh=fp16(x),
    xl=fp16((x-xh)*2^11) etc.  PSUM A accumulates xh*Wh; PSUM B
    accumulates xh*Wl + xl*Wh (both at scale 2^11).  Eviction computes
    S = A + B/2^11 + bias.  Dropped xl*wl term and fp16 rounding are
    ~2^-23 relative: measured end-to-end GEMM err 1.4e-7 — as exact as
    native fp32, so no spike flips vs the fp32 reference.
  - chunk 0 uses k-outer matmul order: each contraction tile is consumed
    in 12 matmuls right when its DMA lands, so compute streams behind the
    (W on sync-queue, x on act-queue) transfers with no bulk wait.
    Later chunks have everything resident and use c-outer for long
    uninterrupted PE runs (keeps the HAM clock gate at full rate).
  - cur^T tiles [no_part, rows], rows t-major/b-minor so the time scan
    reads contiguous 32-lane slices per step.
  - LIF scan: one fused custom-DVE op per timestep:
        mem_{t+1} = (mem_t * 0.9 + cur_t) - (mem_t > 1)
    over [128 part x 32 lanes], lanes = (no_chunk, b).
  - Spike count: (mem_traj > 1) then reduce over t, per chunk; softmax
    without max-subtraction (counts <= 128, use constant shift 64):
    PE-transpose y into one PSUM tile, single Exp eviction, reduce,
    reciprocal, scale.
"""

import numpy as np

B, T, NI, NO = 64, 128, 4096, 512
NCORES = 8
BS = B // NCORES            # 8 batch rows per core
KT = NI // 128              # 32 contraction tiles
CN = NO // 128              # 4 output chunks of 128
TSPLIT = [48, 48, 32]       # timesteps per chunk (last stays 32: its
                            # scan is the exposed tail; bigger earlier
                            # chunks cut matmul-instruction count)
NH = len(TSPLIT)
TOFF = [sum(TSPLIT[:i]) for i in range(NH + 1)]
LAN = CN * BS               # 32 scan lanes per partition
BETA, THR = 0.9, 1.0
SC = float(2.0 ** 11)       # lo-piece scale
SHIFT = 64.0                # softmax constant shift (y in [0,128])
# k-tile DMA grouping: fine-grained leading groups so the first matmuls
# are gated on tiny transfers.
WGRP = [1, 1, 1, 1, 2, 2, 4, 4, 4, 4, 4, 4]
XGRP = [[1, 1, 1, 1, 2, 2, 4, 4, 4, 4, 4, 4]] + [[4] * 8] * (NH - 1)
assert sum(WGRP) == KT and all(sum(g) == KT for g in XGRP)


def _k2g(grp):
    m = {}
    k = 0
    for g, n in enumerate(grp):
        for j in range(n):
            m[k] = (g, j)
            k += 1
    return m


WK2G = _k2g(WGRP)
XK2G = [_k2g(g) for g in XGRP]
# flat x layout: per (chunk h, group g) a [128, n_g*2*RH_h] fp16 block,
# row-major; within a group, k-tile j's hi piece at (2j)*rh, lo at (2j+1)*rh
XSIZES = [128 * n * 2 * (TSPLIT[h] * BS) for h in range(NH) for n in XGRP[h]]
XOFFS = [sum(XSIZES[:i]) for i in range(len(XSIZES) + 1)]
XTOT = XOFFS[-1]
XBLK = [sum(len(XGRP[i]) for i in range(h)) for h in range(NH)]

_PROG = None


def _lif_op():
    """Register (idempotently) the fused LIF-step custom DVE op:
    out = (Src0 * C0 + Src1) - (Src0 > C1)."""
    from concourse import dve_ops
    from concourse.dve_ops import DveOp
    from concourse.dve_spec import Spec, Src0, Src1, C0, C1, lower, _has_src1
    from concourse.dve_uop import DveOpSpec

    name = "LIF_STEP_ANT"
    for op in dve_ops.OPS:
        if op.name == name:
            return op

    spec = Spec(
        body=(Src0 * C0 + Src1) - (Src0 > C1),
        reference=lambda in0, in1, s0, s1, imm2: (
            (in0.astype(np.float32) * np.float32(s0) + in1)
            - (in0 > s1).astype(np.float32)
        ),
    )
    row = dve_ops._CUSTOM_DVE_ROW_BASE + len(dve_ops.OPS)
    assert row < 0x20, "custom DVE opcode rows exhausted"
    dve_ops._SUB_OPCODE_FOR_NAME[name] = row
    shas = {}
    for ver in ("v3", "v4"):
        uops = lower(spec, ver=ver)
        shas[ver] = DveOpSpec(
            name=name, opcode=row, uops=uops, rd1_en=_has_src1(spec)
        ).sha(ver)
    op = DveOp(name, spec, subdim=False, uops_sha=shas)
    dve_ops.OPS.append(op)
    dve_ops.CUSTOM_DVE_SPECS[name] = spec
    return op


def build_program():
    import concourse.bacc as bacc
    import concourse.mybir as mybir
    from concourse import tile
    from concourse.masks import make_identity

    f32 = mybir.dt.float32
    f16 = mybir.dt.float16
    lif = _lif_op()

    nc = bacc.Bacc("TRN2", target_bir_lowering=False, debug=False)

    xt_d = nc.dram_tensor("xt", [XTOT], f16, kind="ExternalInput").ap()
    # per k-tile: Wh_k [128, NO] then Wl_k [128, NO]
    wt_d = nc.dram_tensor("wt", [128, KT * 2 * NO], f16,
                          kind="ExternalInput").ap()
    bt_d = nc.dram_tensor("bt", [CN, 128], f32, kind="ExternalInput").ap()
    out_d = nc.dram_tensor("out", [BS, NO], f32, kind="ExternalOutput").ap()

    def x_dram(h, g):
        i = XBLK[h] + g
        return xt_d[XOFFS[i]:XOFFS[i + 1]].rearrange("(p q) -> p q", p=128)

    with tile.TileContext(nc) as tc:
        with (
            tc.tile_pool(name="wp", bufs=1) as wp,
            tc.tile_pool(name="xp", bufs=12) as xp,
            tc.tile_pool(name="sp", bufs=1) as sp,
            tc.tile_pool(name="cp", bufs=1) as cp,
            tc.tile_pool(name="tmp", bufs=2) as tmpp,
            tc.tile_pool(name="accp", bufs=1, space="PSUM") as accp,
        ):
            # --- W (sync queue) / x (act queue) DMAs --------------------
            wg = [wp.tile([128, n * 2 * NO], f16, name=f"wg{g}", tag=f"wg{g}")
                  for g, n in enumerate(WGRP)]
            xg_tiles = {}

            def issue_x(h, g):
                n = XGRP[h][g]
                t_ = xp.tile([128, n * 2 * TSPLIT[h] * BS], f16,
                             name=f"xg{h}_{g}", tag="xg",
                             padded_shape=[128, 4 * 2 * max(TSPLIT) * BS])
                nc.scalar.dma_start(out=t_[:], in_=x_dram(h, g))
                xg_tiles[(h, g)] = t_

            WOFF = [sum(WGRP[:i]) for i in range(len(WGRP))]

            def issue_w(g):
                nc.sync.dma_start(
                    out=wg[g][:],
                    in_=wt_d[:, WOFF[g] * 2 * NO:(WOFF[g] + WGRP[g]) * 2 * NO])

            for g in range(max(len(WGRP), len(XGRP[0]))):
                if g < len(WGRP):
                    issue_w(g)
                if g < len(XGRP[0]):
                    issue_x(0, g)

            # bias as one value per partition: b_sb[c][p] = b[c*128+p]
            b_sb = [cp.tile([128, 1], f32, name=f"bsb{c}", tag=f"bsb{c}")
                    for c in range(CN)]
            for c in range(CN):
                nc.sync.dma_start(out=b_sb[c][:],
                                  in_=bt_d[c:c + 1, :].rearrange("a p -> p a"))
            zer = cp.tile([128, LAN], f32, name="zer", tag="zer")
            nc.gpsimd.memset(zer[:], 0.0)
            ident = cp.tile([128, 128], f32, name="ident", tag="ident")
            make_identity(nc, ident[:])

            S = [sp.tile([128, TSPLIT[h] * LAN], f32, name=f"s{h}", tag=f"s{h}")
                 for h in range(NH)]
            M = [sp.tile([128, TSPLIT[h] * LAN], f32, name=f"m{h}", tag=f"m{h}")
                 for h in range(NH)]
            yh = [cp.tile([128, LAN], f32, name=f"yh{h}", tag=f"yh{h}")
                  for h in range(NH)]
            ysum = cp.tile([128, LAN], f32, name="ysum", tag="ysum")

            def w_ap(wgt, wj, piece, c):
                off = wj * 2 * NO + piece * NO + c * 128
                return wgt[:, off:off + 128]

            def x_ap(xt_t, xj, piece, rh):
                off = (2 * xj + piece) * rh
                return xt_t[:, off:off + rh]

            # --- GEMM + scan + count, pipelined over time chunks --------
            for h in range(NH):
                ts = TSPLIT[h]
                rh = ts * BS
                # prefetch next chunk's x groups
                if h + 1 < NH:
                    for g in range(len(XGRP[h + 1])):
                        issue_x(h + 1, g)
                accA = [accp.tile([128, rh], f32, name=f"accA{h}_{c}",
                                  tag=f"accA{c}", padded_shape=[128, 512])
                        for c in range(CN)]
                accB = [accp.tile([128, rh], f32, name=f"accB{h}_{c}",
                                  tag=f"accB{c}", padded_shape=[128, 512])
                        for c in range(CN)]

                def mm3(c, k):
                    xgi, xj = XK2G[h][k]
                    wgi, wj = WK2G[k]
                    xt_t = xg_tiles[(h, xgi)]
                    wgt = wg[wgi]
                    xh_ap = x_ap(xt_t, xj, 0, rh)
                    xl_ap = x_ap(xt_t, xj, 1, rh)
                    nc.tensor.matmul(
                        accA[c][:], lhsT=w_ap(wgt, wj, 0, c), rhs=xh_ap,
                        start=(k == 0), stop=(k == KT - 1))
                    nc.tensor.matmul(
                        accB[c][:], lhsT=w_ap(wgt, wj, 0, c), rhs=xl_ap,
                        start=(k == 0), stop=False)
                    nc.tensor.matmul(
                        accB[c][:], lhsT=w_ap(wgt, wj, 1, c), rhs=xh_ap,
                        start=False, stop=(k == KT - 1))

                if h == 0:
                    # W/x stream in during chunk 0: k-outer consumes each
                    # k-tile in 12 matmuls right as its DMA lands, so the
                    # PE tracks the transfers instead of bulk-waiting.
                    for k in range(KT):
                        for c in range(CN):
                            mm3(c, k)
                else:
                    # everything resident: c-outer gives long PE runs
                    for c in range(CN):
                        for k in range(KT):
                            mm3(c, k)
                # evict: S[., ., c] = A + B/SC + bias_c
                s_v = S[h].rearrange("p (t l) -> p t l", l=LAN)
                for c in range(CN):
                    tmp = tmpp.tile([128, rh], f32, name=f"tmp{h}_{c}",
                                    tag="tmp", padded_shape=[128, 512])
                    nc.scalar.activation(
                        tmp[:], accB[c][:],
                        mybir.ActivationFunctionType.Identity,
                        bias=b_sb[c][:], scale=1.0 / SC)
                    nc.vector.tensor_tensor(
                        out=s_v[:, :, c * BS:(c + 1) * BS],
                        in0=accA[c][:].rearrange("p (t b) -> p t b", b=BS),
                        in1=tmp[:].rearrange("p (t b) -> p t b", b=BS),
                        op=mybir.AluOpType.add)
                # LIF scan for this chunk's timesteps
                for tt in range(ts):
                    t = TOFF[h] + tt
                    cur = S[h][:, tt * LAN:(tt + 1) * LAN]
                    dst = M[h][:, tt * LAN:(tt + 1) * LAN]
                    if t == 0:
                        prev = zer[:]
                    elif tt == 0:
                        prev = M[h - 1][:, (TSPLIT[h - 1] - 1) * LAN:
                                        TSPLIT[h - 1] * LAN]
                    else:
                        prev = M[h][:, (tt - 1) * LAN: tt * LAN]
                    nc.vector._custom_dve(lif, out=dst, in0=prev, in1=cur,
                                          s0=BETA, s1=THR)
                # spike count for this chunk (overlaps next chunk's GEMM);
                # S[h]/M[h] are dead after the scan -> reuse as scratch.
                # Contiguous halving tree over t beats the strided reduce
                # (t-major layout pairs equal lanes at each halving).
                nc.vector.tensor_scalar(out=S[h][:], in0=M[h][:], scalar1=THR,
                                        scalar2=None, op0=mybir.AluOpType.is_gt)
                srcs = [S[h], M[h]]
                n = ts * LAN
                si = 0
                while n > LAN and (n // 2) % LAN == 0:
                    half = n // 2
                    a, b = srcs[si], srcs[1 - si]
                    dst = yh[h][:] if half == LAN else b[:, 0:half]
                    nc.vector.tensor_tensor(out=dst, in0=a[:, 0:half],
                                            in1=a[:, half:n],
                                            op=mybir.AluOpType.add)
                    n = half
                    si = 1 - si
                if n > LAN:
                    # odd slice count (non-power-of-2 ts): fold remaining
                    a = srcs[si]
                    nc.vector.tensor_tensor(out=yh[h][:], in0=a[:, 0:LAN],
                                            in1=a[:, LAN:2 * LAN],
                                            op=mybir.AluOpType.add)
                    for j in range(2, n // LAN):
                        nc.vector.tensor_tensor(
                            out=yh[h][:], in0=yh[h][:],
                            in1=a[:, j * LAN:(j + 1) * LAN],
                            op=mybir.AluOpType.add)
                # incremental y accumulation (hidden under later GEMM)
                if h == 1:
                    nc.vector.tensor_tensor(
                        out=ysum[:], in0=yh[0][:], in1=yh[1][:],
                        op=mybir.AluOpType.add)
                elif h >= 2:
                    nc.vector.tensor_tensor(
                        out=ysum[:], in0=ysum[:], in1=yh[h][:],
                        op=mybir.AluOpType.add)

            # --- transpose y^T [no, b] -> one PSUM bank [b, no] ---------
            # (reuses the accA0 bank; each 128-col slice written once, so
            # later start-flag clears don't disturb earlier values)
            ytile = accp.tile([128, 512], f32, name="ytp", tag="accA0")
            ytp = ytile[0:BS, 0:NO]
            y_v = ysum.rearrange("p (c b) -> p c b", b=BS)
            for c in range(CN):
                nc.tensor.transpose(ytp[:, c * 128:(c + 1) * 128],
                                    y_v[:, c, :], ident[:])

            # --- softmax over no (free dim); no max needed: y in [0,128],
            # softmax(y) == softmax(y - SHIFT) exactly ---------------------
            nsh = cp.tile([BS, 1], f32, name="nsh", tag="nsh")
            nc.gpsimd.memset(nsh[:], -SHIFT)
            ex = cp.tile([BS, NO], f32, name="ex", tag="ex")
            nc.scalar.activation(ex[:], ytp[:],
                                 mybir.ActivationFunctionType.Exp,
                                 bias=nsh[:], scale=1.0)
            sm = cp.tile([BS, 1], f32, name="sm", tag="sm")
            nc.vector.tensor_reduce(out=sm[:], in_=ex[:],
                                    axis=mybir.AxisListType.X,
                                    op=mybir.AluOpType.add)
            rc = cp.tile([BS, 1], f32, name="rc", tag="rc")
            nc.vector.reciprocal(rc[:], sm[:])
            res = cp.tile([BS, NO], f32, name="res", tag="res")
            nc.vector.tensor_scalar(out=res[:], in0=ex[:], scalar1=rc[:],
                                    scalar2=None, op0=mybir.AluOpType.mult)

            nc.sync.dma_start(out=out_d[:], in_=res[:])

    nc.compile()
    return nc


def prep_inputs(x, W, b):
    """Host-side layout prep. Returns per-core in_maps."""
    x = np.asarray(x, dtype=np.float32)
    W = np.asarray(W, dtype=np.float32)
    b = np.asarray(b, dtype=np.float32)

    # fp16 hi/lo splits (lo pre-scaled by SC; exact residuals)
    Wh = W.astype(np.float16)
    Wl = ((W - Wh.astype(np.float32)) * SC).astype(np.float16)
    xh = x.astype(np.float16)
    xl = ((x - xh.astype(np.float32)) * SC).astype(np.float16)

    # wbig[p, k*2*NO + piece*NO + j] = Wpiece[j, k*128 + p]
    def wlayout(Wp):
        return np.ascontiguousarray(
            Wp.T.reshape(KT, 128, NO).transpose(1, 0, 2))  # [128, KT, NO]

    wb = np.empty((128, KT, 2, NO), np.float16)
    wb[:, :, 0, :] = wlayout(Wh)
    wb[:, :, 1, :] = wlayout(Wl)
    wbig = np.ascontiguousarray(wb.reshape(128, KT * 2 * NO))
    bt = np.ascontiguousarray(b.reshape(CN, 128))

    # x flat layout: blocks (h, g) of [128, n_g, 2, RH_h], r = tt*BS + b_loc
    xTh = np.ascontiguousarray(xh.transpose(2, 1, 0))  # [NI, T, B] fp16
    xTl = np.ascontiguousarray(xl.transpose(2, 1, 0))
    in_maps = []
    for ci in range(NCORES):
        bsl = slice(ci * BS, (ci + 1) * BS)
        xkh = xTh[:, :, bsl].reshape(KT, 128, T * BS)   # [k, p, t*BS+b]
        xkl = xTl[:, :, bsl].reshape(KT, 128, T * BS)
        flat = np.empty(XTOT, np.float16)
        i = 0
        for h in range(NH):
            r0, r1 = TOFF[h] * BS, TOFF[h + 1] * BS
            rh = r1 - r0
            k0 = 0
            for n_g in XGRP[h]:
                blk = np.empty((128, n_g, 2, rh), np.float16)
                blk[:, :, 0, :] = xkh[k0:k0 + n_g, :, r0:r1].transpose(1, 0, 2)
                blk[:, :, 1, :] = xkl[k0:k0 + n_g, :, r0:r1].transpose(1, 0, 2)
                flat[XOFFS[i]:XOFFS[i] + blk.size] = blk.reshape(-1)
                k0 += n_g
                i += 1
        in_maps.append({"xt": flat, "wt": wbig, "bt": bt})
    return in_maps


def get_program():
    global _PROG
    if _PROG is None:
        _PROG = build_program()
    return _PROG


def kernel(x, W, b):
    from concourse import bass_utils

    nc = get_program()
    in_maps = prep_inputs(x, W, b)
    res = bass_utils.run_bass_kernel_spmd(nc, in_maps,
                                          core_ids=list(range(NCORES)))
    return np.concatenate([res.results[i]["out"] for i in range(NCORES)],
                          axis=0)
